# revision 3
# baseline (speedup 1.0000x reference)
"""Trainium2 Bass kernel for nn_CrossAttentionModule (B=4, C=2048, H=W=32).

The module is two independent cross-attention streams per batch element
(RGB queries over index features, and index queries over RGB features).
That yields 8 perfectly independent units = 4 batches x 2 streams; one
unit per NeuronCore, zero collectives.

Per-core program (all matmuls bf16, fp32 PSUM accumulate):
  Q  = (Wq/sqrt(C)) @ Xq + bq/sqrt(C)        [C, N]   (scale folded on host)
  K  = Wk @ Xkv + bk                          [C, N]
  VT = Xkv^T @ Wv^T + 1 x bv                  [N, C]   (computed directly
       transposed; bias added as a K=1 rank-1 matmul into the same PSUM;
       weights streamed as small kt-slabs against 8 parallel PSUM chains)
  S  = Q^T K                                  [N, N]
  A  = softmax(S, axis=-1)  (row max via negate-reduce, Exp activation with
       fused accum row-sum, reciprocal, row scale)
  AT = A^T via DMA transposes (128x128 bf16 blocks, runs on idle DMA engines)
  O  = (VT)^T @ AT = V @ A^T                  [C, N]  fp32 out

Host side: pre-transposes/pre-tiles the weights into the exact slab layout
the kernel streams (every DMA is contiguous), casts to bf16, distributes
the 8 units across cores, and reassembles the 4 reference outputs.
"""

import math
from functools import lru_cache

import ml_dtypes
import numpy as np

B, C, HW, N = 4, 2048, 32, 1024
P = 128
CT = C // P           # 16 channel tiles
NT = N // P           # 8 pixel tiles
KHALF = 512           # moving free dim per matmul
CG = C // KHALF       # 4 output-channel groups for the VT conv

_BF16 = ml_dtypes.bfloat16


def _build_program():
    import concourse.bass as bass
    import concourse.mybir as mybir
    import concourse.tile as tile
    from concourse import bacc
    from concourse.masks import make_identity

    dtb = mybir.dt.bfloat16
    dtf = mybir.dt.float32

    nc = bacc.Bacc("TRN2", target_bir_lowering=False, debug=False)

    xq_d = nc.declare_dram_parameter("xq", [C, N], dtb, isOutput=False)
    xkv_d = nc.declare_dram_parameter("xkv", [C, N], dtb, isOutput=False)
    wq_d = nc.declare_dram_parameter("wq", [CT, P, CT, P], dtb, isOutput=False)
    wk_d = nc.declare_dram_parameter("wk", [CT, P, CT, P], dtb, isOutput=False)
    # wv tiled as [cg, kt, ci, co]: kt-slab (cg, kt) = [128 ci, 512 co] contiguous
    wv_d = nc.declare_dram_parameter("wv", [CG, CT, P, KHALF], dtb, isOutput=False)
    bq_d = nc.declare_dram_parameter("bq", [P, CT], dtf, isOutput=False)
    bk_d = nc.declare_dram_parameter("bk", [P, CT], dtf, isOutput=False)
    bv_d = nc.declare_dram_parameter("bv", [1, C], dtb, isOutput=False)
    out_d = nc.declare_dram_parameter("out", [C, N], dtf, isOutput=True)

    with tile.TileContext(nc) as tc:
        with (
            tc.tile_pool(name="const", bufs=1) as const_pool,
            tc.tile_pool(name="big", bufs=1) as big_pool,
            tc.tile_pool(name="wqk", bufs=3) as wqk_pool,
            tc.tile_pool(name="wv", bufs=4) as wv_pool,
            tc.tile_pool(name="stat", bufs=8) as stat_pool,
            tc.tile_pool(name="ostage", bufs=3) as ostage_pool,
        ):
            # constants
            ones_row = const_pool.tile([1, P], dtb)
            nc.gpsimd.memset(ones_row[:], 1.0)
            bq_sb = const_pool.tile([P, CT], dtf)
            nc.sync.dma_start(bq_sb[:], bq_d[:])
            bk_sb = const_pool.tile([P, CT], dtf)
            nc.sync.dma_start(bk_sb[:], bk_d[:])
            bv_sb = const_pool.tile([1, C], dtb)
            nc.sync.dma_start(bv_sb[:], bv_d[:])

            # persistent activations
            q_sb = big_pool.tile([P, CT, N], dtb)     # Q[c, n]
            k_sb = big_pool.tile([P, CT, N], dtb)     # K[c, n]
            vt_sb = big_pool.tile([P, NT, C], dtb)    # V^T[m, c]

            with (
                tc.tile_pool(name="x", bufs=1) as x_pool,
                tc.tile_pool(name="psconv", bufs=8, space=bass.MemorySpace.PSUM)
                as psconv,
            ):
                xq_sb = x_pool.tile([P, CT, N], dtb)
                xkv_sb = x_pool.tile([P, CT, N], dtb)

                # First weight slab ahead of the activations so the first
                # matmul isn't queued behind the full 4MB xq transfer.
                w_slab0 = wqk_pool.tile([P, CT, P], dtb, tag="wslab")
                nc.sync.dma_start(w_slab0[:], wq_d[0])
                for kt in range(CT):
                    nc.sync.dma_start(xq_sb[:, kt, :], xq_d[kt * P : (kt + 1) * P, :])
                for kt in range(CT):
                    nc.sync.dma_start(
                        xkv_sb[:, kt, :], xkv_d[kt * P : (kt + 1) * P, :]
                    )

                def conv_qk(x_sb, w_dram, b_sb, dst, slab0=None):
                    for ot in range(CT):
                        if ot == 0 and slab0 is not None:
                            w_slab = slab0
                        else:
                            w_slab = wqk_pool.tile([P, CT, P], dtb, tag="wslab")
                            nc.sync.dma_start(w_slab[:], w_dram[ot])
                        ps0 = psconv.tile([P, KHALF], dtf, tag="mm")
                        ps1 = psconv.tile([P, KHALF], dtf, tag="mm")
                        for kt in range(CT):
                            nc.tensor.matmul(
                                ps0[:], w_slab[:, kt, :], x_sb[:, kt, 0:KHALF],
                                start=(kt == 0), stop=(kt == CT - 1),
                            )
                            nc.tensor.matmul(
                                ps1[:], w_slab[:, kt, :], x_sb[:, kt, KHALF:N],
                                start=(kt == 0), stop=(kt == CT - 1),
                            )
                        nc.vector.tensor_scalar_add(
                            dst[:, ot, 0:KHALF], ps0[:], b_sb[:, ot : ot + 1]
                        )
                        nc.vector.tensor_scalar_add(
                            dst[:, ot, KHALF:N], ps1[:], b_sb[:, ot : ot + 1]
                        )

                conv_qk(xq_sb, wq_d, bq_sb, q_sb, slab0=w_slab0)
                conv_qk(xkv_sb, wk_d, bk_sb, k_sb)

                # VT conv: VT[m, c] = sum_ci Xkv[ci, m] WvT[ci, c] + bv[c].
                # Weight kt-slab [128, 512] streams once; 8 m-tile PSUM
                # chains accumulate in parallel so every slab is consumed
                # immediately by 8 matmuls.
                for cg in range(CG):
                    chains = []
                    for _mt in range(NT):
                        ch = psconv.tile([P, KHALF], dtf, tag="mm", name=f"vt{cg}_{_mt}")
                        chains.append(ch)
                    for kt in range(CT):
                        wslab = wv_pool.tile([P, KHALF], dtb, tag="wv")
                        nc.sync.dma_start(wslab[:], wv_d[cg, kt])
                        for mt in range(NT):
                            nc.tensor.matmul(
                                chains[mt][:],
                                xkv_sb[:, kt, mt * P : (mt + 1) * P],
                                wslab[:],
                                start=(kt == 0), stop=False,
                            )
                    for mt in range(NT):
                        nc.tensor.matmul(
                            chains[mt][:],
                            ones_row[:],
                            bv_sb[:, cg * KHALF : (cg + 1) * KHALF],
                            start=False, stop=True,
                        )
                        nc.vector.tensor_copy(
                            vt_sb[:, mt, cg * KHALF : (cg + 1) * KHALF],
                            chains[mt][:],
                        )

            # ---- S = Q^T K, softmax, A^T (DMA transpose), O = V A^T ----
            with (
                tc.tile_pool(name="attn", bufs=1) as attn_pool,
                tc.tile_pool(name="pss", bufs=2, space=bass.MemorySpace.PSUM) as pss,
                tc.tile_pool(name="pso", bufs=3, space=bass.MemorySpace.PSUM) as pso,
            ):
                a_sb = attn_pool.tile([P, NT, N], dtb)   # A[nq, nk]
                at_sb = attn_pool.tile([P, NT, N], dtb)  # A^T[m, nq]

                for qt in range(NT):
                    ps = pss.tile([P, N], dtf, tag="s")  # two banks
                    for nh in range(2):
                        for kt in range(CT):
                            nc.tensor.matmul(
                                ps[:, nh * KHALF : (nh + 1) * KHALF],
                                q_sb[:, kt, qt * P : (qt + 1) * P],
                                k_sb[:, kt, nh * KHALF : (nh + 1) * KHALF],
                                start=(kt == 0), stop=(kt == CT - 1),
                            )
                    nmax = stat_pool.tile([P, 1], dtf, tag="nmax")
                    nc.vector.reduce_max(
                        nmax[:], ps[:], axis=mybir.AxisListType.X, negate=True
                    )
                    rsum = stat_pool.tile([P, 1], dtf, tag="rsum")
                    nc.scalar.activation(
                        a_sb[:, qt, :], ps[:],
                        mybir.ActivationFunctionType.Exp,
                        bias=nmax[:], scale=1.0, accum_out=rsum[:],
                    )
                    rinv = stat_pool.tile([P, 1], dtf, tag="rinv")
                    nc.vector.reciprocal(rinv[:], rsum[:])
                    nc.vector.tensor_scalar_mul(
                        a_sb[:, qt, :], a_sb[:, qt, :], rinv[:]
                    )
                    # transpose this row-block of A into AT's column blocks
                    # on the (otherwise idle) DMA engines
                    for mt in range(NT):
                        nc.sync.dma_start(
                            at_sb[:, mt, qt * P : (qt + 1) * P],
                            a_sb[:, qt, mt * P : (mt + 1) * P],
                            transpose=True,
                        )

                # O = V @ A^T, one n-half at a time (the nh=1 half needs the
                # last row-blocks' transposes, which finish during nh=0)
                for nh in range(2):
                    for ct in range(CT):
                        ps = pso.tile([P, KHALF], dtf, tag="o")
                        for mt in range(NT):
                            nc.tensor.matmul(
                                ps[:],
                                vt_sb[:, mt, ct * P : (ct + 1) * P],
                                at_sb[:, mt, nh * KHALF : (nh + 1) * KHALF],
                                start=(mt == 0), stop=(mt == NT - 1),
                            )
                        o_stage = ostage_pool.tile([P, KHALF], dtf, tag="o")
                        nc.vector.tensor_copy(o_stage[:], ps[:])
                        nc.sync.dma_start(
                            out_d[ct * P : (ct + 1) * P, nh * KHALF : (nh + 1) * KHALF],
                            o_stage[:],
                        )

    nc.compile()
    return nc


@lru_cache(maxsize=1)
def _get_nc():
    return _build_program()


def _prep_wqk(W, b, scale):
    WT = np.ascontiguousarray(W.T) * scale  # [c_in, c_out]
    wt = np.ascontiguousarray(
        WT.reshape(CT, P, CT, P).transpose(2, 1, 0, 3)
    ).astype(_BF16)  # [ot, ci, kt, o]
    bp = np.ascontiguousarray((b * scale).reshape(CT, P).T).astype(np.float32)
    return wt, bp


def _prep_wv(W, b):
    WT = np.ascontiguousarray(W.T)  # [c_in, c_out]
    wt = np.ascontiguousarray(
        WT.reshape(CT, P, CG, KHALF).transpose(2, 0, 1, 3)
    ).astype(_BF16)  # [cg, kt, ci, co]
    bv = np.ascontiguousarray(b.reshape(1, C)).astype(_BF16)
    return wt, bv


def _run(inputs, trace=False):
    from concourse.bass_utils import run_bass_kernel_spmd

    F_rgb = np.asarray(inputs["F_rgb"], dtype=np.float32)
    F_ind = np.asarray(inputs["F_indices"], dtype=np.float32)

    scale = 1.0 / math.sqrt(C)
    # stream 0: rgb queries attend over index features
    wq0, bq0 = _prep_wqk(np.asarray(inputs["W_q_rgb"], np.float32),
                         np.asarray(inputs["b_q_rgb"], np.float32), scale)
    wk0, bk0 = _prep_wqk(np.asarray(inputs["W_k_ind"], np.float32),
                         np.asarray(inputs["b_k_ind"], np.float32), 1.0)
    wv0, bv0 = _prep_wv(np.asarray(inputs["W_v_ind"], np.float32),
                        np.asarray(inputs["b_v_ind"], np.float32))
    # stream 1: index queries attend over rgb features
    wq1, bq1 = _prep_wqk(np.asarray(inputs["W_q_ind"], np.float32),
                         np.asarray(inputs["b_q_ind"], np.float32), scale)
    wk1, bk1 = _prep_wqk(np.asarray(inputs["W_k_rgb"], np.float32),
                         np.asarray(inputs["b_k_rgb"], np.float32), 1.0)
    wv1, bv1 = _prep_wv(np.asarray(inputs["W_v_rgb"], np.float32),
                        np.asarray(inputs["b_v_rgb"], np.float32))

    rgb_flat = [np.ascontiguousarray(F_rgb[b].reshape(C, N)).astype(_BF16)
                for b in range(B)]
    ind_flat = [np.ascontiguousarray(F_ind[b].reshape(C, N)).astype(_BF16)
                for b in range(B)]

    in_maps = []
    for b in range(B):  # cores 0-3: stream 0
        in_maps.append(dict(xq=rgb_flat[b], xkv=ind_flat[b], wq=wq0, wk=wk0,
                            wv=wv0, bq=bq0, bk=bk0, bv=bv0))
    for b in range(B):  # cores 4-7: stream 1
        in_maps.append(dict(xq=ind_flat[b], xkv=rgb_flat[b], wq=wq1, wk=wk1,
                            wv=wv1, bq=bq1, bk=bk1, bv=bv1))

    nc = _get_nc()
    res = run_bass_kernel_spmd(nc, in_maps, core_ids=list(range(8)), trace=trace)

    O1 = np.stack([res.results[b]["out"].reshape(C, HW, HW) for b in range(B)])
    O2 = np.stack([res.results[4 + b]["out"].reshape(C, HW, HW) for b in range(B)])
    F_final = O1 + O2
    attention_weights = np.stack([O1, O2], axis=1)
    return (F_final, F_rgb, F_ind, attention_weights), res


def kernel(**inputs):
    outs, _ = _run(inputs, trace=False)
    return outs


def kernel_profiled(**inputs):
    outs, res = _run(inputs, trace=True)
    return outs, res


# revision 4
# speedup vs baseline: 1.0493x; 1.0493x over previous
"""Trainium2 Bass kernel for nn_CrossAttentionModule (B=4, C=2048, H=W=32).

The module is two independent cross-attention streams per batch element
(RGB queries over index features, and index queries over RGB features).
That yields 8 perfectly independent units = 4 batches x 2 streams; one
unit per NeuronCore, zero collectives.

Per-core program (all matmuls bf16, fp32 PSUM accumulate):
  Q  = (Wq/sqrt(C)) @ Xq + bq/sqrt(C)        [C, N]   (scale folded on host)
  K  = Wk @ Xkv + bk                          [C, N]
  VT = Xkv^T @ Wv^T + 1 x bv                  [N, C]   (computed directly
       transposed; bias added as a K=1 rank-1 matmul into the same PSUM;
       weights streamed as small kt-slabs against 8 parallel PSUM chains)
  ST = K^T Q                                  [N(key), N(query)] -- scores
       computed TRANSPOSED so the contraction index of the subsequent
       O-matmul (the key index m) lands on the partition dim: no transpose
       of the attention matrix is ever needed.
  E  = exp(ST)   (no max subtraction: |scores| <~ 5, exp is safe in fp32;
       softmax ratios are mathematically identical)
  colsum_j = sum_m E[m, j] via a ones-vector PE matmul (partition reduce)
  O  = (V E^T) * (1/colsum broadcast)         [C, N]  fp32 out
       (normalization folded into the O epilogue with gpsimd
       partition_broadcast of the reciprocal row sums)

Host side: pre-transposes/pre-tiles the weights into the exact slab layout
the kernel streams (every DMA is contiguous), casts to bf16, distributes
the 8 units across cores, and reassembles the 4 reference outputs.
"""

import math
from functools import lru_cache

import ml_dtypes
import numpy as np

B, C, HW, N = 4, 2048, 32, 1024
P = 128
CT = C // P           # 16 channel tiles
NT = N // P           # 8 pixel tiles
KHALF = 512           # moving free dim per matmul
CG = C // KHALF       # 4 output-channel groups for the VT conv

_BF16 = ml_dtypes.bfloat16


def _build_program():
    import concourse.bass as bass
    import concourse.mybir as mybir
    import concourse.tile as tile
    from concourse import bacc

    dtb = mybir.dt.bfloat16
    dtf = mybir.dt.float32

    nc = bacc.Bacc("TRN2", target_bir_lowering=False, debug=False)

    xq_d = nc.declare_dram_parameter("xq", [C, N], dtb, isOutput=False)
    xkv_d = nc.declare_dram_parameter("xkv", [C, N], dtb, isOutput=False)
    wq_d = nc.declare_dram_parameter("wq", [CT, P, CT, P], dtb, isOutput=False)
    wk_d = nc.declare_dram_parameter("wk", [CT, P, CT, P], dtb, isOutput=False)
    # wv tiled as [cg, kt, ci, co]: kt-slab (cg, kt) = [128 ci, 512 co] contiguous
    wv_d = nc.declare_dram_parameter("wv", [CG, CT, P, KHALF], dtb, isOutput=False)
    bq_d = nc.declare_dram_parameter("bq", [P, CT], dtf, isOutput=False)
    bk_d = nc.declare_dram_parameter("bk", [P, CT], dtf, isOutput=False)
    bv_d = nc.declare_dram_parameter("bv", [1, C], dtb, isOutput=False)
    out_d = nc.declare_dram_parameter("out", [C, N], dtf, isOutput=True)

    with tile.TileContext(nc) as tc:
        with (
            tc.tile_pool(name="const", bufs=1) as const_pool,
            tc.tile_pool(name="big", bufs=1) as big_pool,
            tc.tile_pool(name="wqk", bufs=3) as wqk_pool,
            tc.tile_pool(name="wv", bufs=4) as wv_pool,
            tc.tile_pool(name="ostage", bufs=3) as ostage_pool,
        ):
            # constants
            ones_row = const_pool.tile([1, P], dtb)
            nc.gpsimd.memset(ones_row[:], 1.0)
            ones_col = const_pool.tile([P, 1], dtb)
            nc.gpsimd.memset(ones_col[:], 1.0)
            bq_sb = const_pool.tile([P, CT], dtf)
            nc.sync.dma_start(bq_sb[:], bq_d[:])
            bk_sb = const_pool.tile([P, CT], dtf)
            nc.sync.dma_start(bk_sb[:], bk_d[:])
            bv_sb = const_pool.tile([1, C], dtb)
            nc.sync.dma_start(bv_sb[:], bv_d[:])

            # persistent activations
            q_sb = big_pool.tile([P, CT, N], dtb)     # Q[c, n]
            k_sb = big_pool.tile([P, CT, N], dtb)     # K[c, n]
            vt_sb = big_pool.tile([P, NT, C], dtb)    # V^T[m, c]

            with (
                tc.tile_pool(name="x", bufs=1) as x_pool,
                tc.tile_pool(name="psconv", bufs=8, space=bass.MemorySpace.PSUM)
                as psconv,
            ):
                xq_sb = x_pool.tile([P, CT, N], dtb)
                xkv_sb = x_pool.tile([P, CT, N], dtb)

                # First weight slab ahead of the activations so the first
                # matmul isn't queued behind the full 4MB xq transfer.
                w_slab0 = wqk_pool.tile([P, CT, P], dtb, tag="wslab")
                nc.sync.dma_start(w_slab0[:], wq_d[0])
                for kt in range(CT):
                    nc.sync.dma_start(xq_sb[:, kt, :], xq_d[kt * P : (kt + 1) * P, :])
                for kt in range(CT):
                    nc.sync.dma_start(
                        xkv_sb[:, kt, :], xkv_d[kt * P : (kt + 1) * P, :]
                    )

                def conv_qk(x_sb, w_dram, b_sb, dst, slab0=None):
                    for ot in range(CT):
                        if ot == 0 and slab0 is not None:
                            w_slab = slab0
                        else:
                            w_slab = wqk_pool.tile([P, CT, P], dtb, tag="wslab")
                            nc.sync.dma_start(w_slab[:], w_dram[ot])
                        ps0 = psconv.tile([P, KHALF], dtf, tag="mm")
                        ps1 = psconv.tile([P, KHALF], dtf, tag="mm")
                        for kt in range(CT):
                            nc.tensor.matmul(
                                ps0[:], w_slab[:, kt, :], x_sb[:, kt, 0:KHALF],
                                start=(kt == 0), stop=(kt == CT - 1),
                            )
                            nc.tensor.matmul(
                                ps1[:], w_slab[:, kt, :], x_sb[:, kt, KHALF:N],
                                start=(kt == 0), stop=(kt == CT - 1),
                            )
                        nc.vector.tensor_scalar_add(
                            dst[:, ot, 0:KHALF], ps0[:], b_sb[:, ot : ot + 1]
                        )
                        nc.vector.tensor_scalar_add(
                            dst[:, ot, KHALF:N], ps1[:], b_sb[:, ot : ot + 1]
                        )

                conv_qk(xq_sb, wq_d, bq_sb, q_sb, slab0=w_slab0)
                conv_qk(xkv_sb, wk_d, bk_sb, k_sb)

                # VT conv: VT[m, c] = sum_ci Xkv[ci, m] WvT[ci, c] + bv[c].
                # Weight kt-slab [128, 512] streams once; 8 m-tile PSUM
                # chains accumulate in parallel so every slab is consumed
                # immediately by 8 matmuls.
                for cg in range(CG):
                    chains = []
                    for _mt in range(NT):
                        ch = psconv.tile(
                            [P, KHALF], dtf, tag="mm", name=f"vt{cg}_{_mt}"
                        )
                        chains.append(ch)
                    for kt in range(CT):
                        wslab = wv_pool.tile([P, KHALF], dtb, tag="wv")
                        nc.sync.dma_start(wslab[:], wv_d[cg, kt])
                        for mt in range(NT):
                            nc.tensor.matmul(
                                chains[mt][:],
                                xkv_sb[:, kt, mt * P : (mt + 1) * P],
                                wslab[:],
                                start=(kt == 0), stop=False,
                            )
                    for mt in range(NT):
                        nc.tensor.matmul(
                            chains[mt][:],
                            ones_row[:],
                            bv_sb[:, cg * KHALF : (cg + 1) * KHALF],
                            start=False, stop=True,
                        )
                        nc.vector.tensor_copy(
                            vt_sb[:, mt, cg * KHALF : (cg + 1) * KHALF],
                            chains[mt][:],
                        )

            # ---- ST = K^T Q, E = exp(ST), colsums, O = (V E^T) / colsum ----
            with (
                tc.tile_pool(name="attn", bufs=1) as attn_pool,
                tc.tile_pool(name="pss", bufs=2, space=bass.MemorySpace.PSUM) as pss,
                tc.tile_pool(name="pssum", bufs=1, space=bass.MemorySpace.PSUM)
                as pssum,
                tc.tile_pool(name="pso", bufs=2, space=bass.MemorySpace.PSUM) as pso,
            ):
                et_sb = attn_pool.tile([P, NT, N], dtb)   # E[m, nq] = exp(S^T)
                rb_sb = attn_pool.tile([P, N], dtf)       # 1/colsum, broadcast
                rinv_sb = attn_pool.tile([1, N], dtf)
                sums_ps = pssum.tile([1, N], dtf)

                for mt in range(NT):
                    ps = pss.tile([P, N], dtf, tag="s")  # two banks
                    for nh in range(2):
                        for kt in range(CT):
                            nc.tensor.matmul(
                                ps[:, nh * KHALF : (nh + 1) * KHALF],
                                k_sb[:, kt, mt * P : (mt + 1) * P],
                                q_sb[:, kt, nh * KHALF : (nh + 1) * KHALF],
                                start=(kt == 0), stop=(kt == CT - 1),
                            )
                    nc.scalar.activation(
                        et_sb[:, mt, :], ps[:],
                        mybir.ActivationFunctionType.Exp,
                    )

                # column sums over the partition (key) dim via ones matmuls
                for mt in range(NT):
                    for nh in range(2):
                        nc.tensor.matmul(
                            sums_ps[:, nh * KHALF : (nh + 1) * KHALF],
                            ones_col[:],
                            et_sb[:, mt, nh * KHALF : (nh + 1) * KHALF],
                            start=(mt == 0), stop=(mt == NT - 1),
                        )
                nc.vector.reciprocal(rinv_sb[:], sums_ps[:])
                nc.gpsimd.partition_broadcast(rb_sb[:], rinv_sb[:])

                # O = V @ E^T, normalized in the epilogue
                for nh in range(2):
                    for ct in range(CT):
                        ps = pso.tile([P, KHALF], dtf, tag="o")
                        for mt in range(NT):
                            nc.tensor.matmul(
                                ps[:],
                                vt_sb[:, mt, ct * P : (ct + 1) * P],
                                et_sb[:, mt, nh * KHALF : (nh + 1) * KHALF],
                                start=(mt == 0), stop=(mt == NT - 1),
                            )
                        o_stage = ostage_pool.tile([P, KHALF], dtf, tag="o")
                        nc.vector.tensor_mul(
                            o_stage[:], ps[:], rb_sb[:, nh * KHALF : (nh + 1) * KHALF]
                        )
                        nc.sync.dma_start(
                            out_d[ct * P : (ct + 1) * P, nh * KHALF : (nh + 1) * KHALF],
                            o_stage[:],
                        )

    nc.compile()
    return nc


@lru_cache(maxsize=1)
def _get_nc():
    return _build_program()


def _prep_wqk(W, b, scale):
    WT = np.ascontiguousarray(W.T) * scale  # [c_in, c_out]
    wt = np.ascontiguousarray(
        WT.reshape(CT, P, CT, P).transpose(2, 1, 0, 3)
    ).astype(_BF16)  # [ot, ci, kt, o]
    bp = np.ascontiguousarray((b * scale).reshape(CT, P).T).astype(np.float32)
    return wt, bp


def _prep_wv(W, b):
    WT = np.ascontiguousarray(W.T)  # [c_in, c_out]
    wt = np.ascontiguousarray(
        WT.reshape(CT, P, CG, KHALF).transpose(2, 0, 1, 3)
    ).astype(_BF16)  # [cg, kt, ci, co]
    bv = np.ascontiguousarray(b.reshape(1, C)).astype(_BF16)
    return wt, bv


def _run(inputs, trace=False):
    from concourse.bass_utils import run_bass_kernel_spmd

    F_rgb = np.asarray(inputs["F_rgb"], dtype=np.float32)
    F_ind = np.asarray(inputs["F_indices"], dtype=np.float32)

    scale = 1.0 / math.sqrt(C)
    # stream 0: rgb queries attend over index features
    wq0, bq0 = _prep_wqk(np.asarray(inputs["W_q_rgb"], np.float32),
                         np.asarray(inputs["b_q_rgb"], np.float32), scale)
    wk0, bk0 = _prep_wqk(np.asarray(inputs["W_k_ind"], np.float32),
                         np.asarray(inputs["b_k_ind"], np.float32), 1.0)
    wv0, bv0 = _prep_wv(np.asarray(inputs["W_v_ind"], np.float32),
                        np.asarray(inputs["b_v_ind"], np.float32))
    # stream 1: index queries attend over rgb features
    wq1, bq1 = _prep_wqk(np.asarray(inputs["W_q_ind"], np.float32),
                         np.asarray(inputs["b_q_ind"], np.float32), scale)
    wk1, bk1 = _prep_wqk(np.asarray(inputs["W_k_rgb"], np.float32),
                         np.asarray(inputs["b_k_rgb"], np.float32), 1.0)
    wv1, bv1 = _prep_wv(np.asarray(inputs["W_v_rgb"], np.float32),
                        np.asarray(inputs["b_v_rgb"], np.float32))

    rgb_flat = [np.ascontiguousarray(F_rgb[b].reshape(C, N)).astype(_BF16)
                for b in range(B)]
    ind_flat = [np.ascontiguousarray(F_ind[b].reshape(C, N)).astype(_BF16)
                for b in range(B)]

    in_maps = []
    for b in range(B):  # cores 0-3: stream 0
        in_maps.append(dict(xq=rgb_flat[b], xkv=ind_flat[b], wq=wq0, wk=wk0,
                            wv=wv0, bq=bq0, bk=bk0, bv=bv0))
    for b in range(B):  # cores 4-7: stream 1
        in_maps.append(dict(xq=ind_flat[b], xkv=rgb_flat[b], wq=wq1, wk=wk1,
                            wv=wv1, bq=bq1, bk=bk1, bv=bv1))

    nc = _get_nc()
    res = run_bass_kernel_spmd(nc, in_maps, core_ids=list(range(8)), trace=trace)

    O1 = np.stack([res.results[b]["out"].reshape(C, HW, HW) for b in range(B)])
    O2 = np.stack([res.results[4 + b]["out"].reshape(C, HW, HW) for b in range(B)])
    F_final = O1 + O2
    attention_weights = np.stack([O1, O2], axis=1)
    return (F_final, F_rgb, F_ind, attention_weights), res


def kernel(**inputs):
    outs, _ = _run(inputs, trace=False)
    return outs


def kernel_profiled(**inputs):
    outs, res = _run(inputs, trace=True)
    return outs, res


# revision 5
# speedup vs baseline: 1.0732x; 1.0228x over previous
"""Trainium2 Bass kernel for nn_CrossAttentionModule (B=4, C=2048, H=W=32).

The module is two independent cross-attention streams per batch element
(RGB queries over index features, and index queries over RGB features).
That yields 8 perfectly independent units = 4 batches x 2 streams; one
unit per NeuronCore, zero collectives.

Per-core program (all matmuls bf16, fp32 PSUM accumulate):
  Q  = (Wq/sqrt(C)) @ Xq + bq/sqrt(C)        [C, N]   (scale folded on host)
  K  = Wk @ Xkv + bk                          [C, N]
  VT = Xkv^T @ Wv^T + 1 x bv                  [N, C]   (computed directly
       transposed; bias added as a K=1 rank-1 matmul into the same PSUM;
       weights streamed as small kt-slabs against 4 parallel PSUM chains)
  ST = K^T Q                                  [N(key), N(query)] -- scores
       computed TRANSPOSED so the contraction index of the subsequent
       O-matmul (the key index m) lands on the partition dim: no transpose
       of the attention matrix is ever needed.
  E  = exp(ST)   (no max subtraction: |scores| <~ 5, exp is safe in fp32;
       softmax ratios are mathematically identical)
  colsum = ones[128,128]^T @ E  -- partition-reduce on the PE that lands
       the SAME sum on every partition, so 1/colsum needs no broadcast and
       the reciprocal runs wide on the DVE
  O  = (V E^T) * (1/colsum)                   [C, N]  fp32 out

Host side: pre-transposes/pre-tiles the weights into the exact slab layout
the kernel streams (every DMA is contiguous), casts to bf16, distributes
the 8 units across cores, and reassembles the 4 reference outputs.
"""

import math
from functools import lru_cache

import ml_dtypes
import numpy as np

B, C, HW, N = 4, 2048, 32, 1024
P = 128
CT = C // P           # 16 channel tiles
NT = N // P           # 8 pixel tiles
KHALF = 512           # moving free dim per matmul
CG = C // KHALF       # 4 output-channel groups for the VT conv

_BF16 = ml_dtypes.bfloat16


def _build_program():
    import concourse.bass as bass
    import concourse.mybir as mybir
    import concourse.tile as tile
    from concourse import bacc

    dtb = mybir.dt.bfloat16
    dtf = mybir.dt.float32

    nc = bacc.Bacc("TRN2", target_bir_lowering=False, debug=False)

    xq_d = nc.declare_dram_parameter("xq", [C, N], dtb, isOutput=False)
    xkv_d = nc.declare_dram_parameter("xkv", [C, N], dtb, isOutput=False)
    wq_d = nc.declare_dram_parameter("wq", [CT, P, CT, P], dtb, isOutput=False)
    wk_d = nc.declare_dram_parameter("wk", [CT, P, CT, P], dtb, isOutput=False)
    # wv tiled as [cg, kt, ci, co]: kt-slab (cg, kt) = [128 ci, 512 co] contiguous
    wv_d = nc.declare_dram_parameter("wv", [CG, CT, P, KHALF], dtb, isOutput=False)
    bq_d = nc.declare_dram_parameter("bq", [P, CT], dtf, isOutput=False)
    bk_d = nc.declare_dram_parameter("bk", [P, CT], dtf, isOutput=False)
    bv_d = nc.declare_dram_parameter("bv", [1, C], dtb, isOutput=False)
    out_d = nc.declare_dram_parameter("out", [C, N], dtf, isOutput=True)

    with tile.TileContext(nc) as tc:
        with (
            tc.tile_pool(name="const", bufs=1) as const_pool,
            tc.tile_pool(name="big", bufs=1) as big_pool,
            tc.tile_pool(name="wqk", bufs=3) as wqk_pool,
            tc.tile_pool(name="wv", bufs=4) as wv_pool,
            tc.tile_pool(name="ostage", bufs=3) as ostage_pool,
            # scores PSUM lives OUTSIDE the conv scope so its banks never
            # overlap the conv pool's -- the first ST matmul then has no
            # wait on the VT epilogue drain
            tc.tile_pool(name="pss", bufs=2, space=bass.MemorySpace.PSUM) as pss,
        ):
            # constants
            ones_row = const_pool.tile([1, P], dtb)
            nc.gpsimd.memset(ones_row[:], 1.0)
            ones128 = const_pool.tile([P, P], dtb)
            nc.gpsimd.memset(ones128[:], 1.0)
            bq_sb = const_pool.tile([P, CT], dtf)
            bk_sb = const_pool.tile([P, CT], dtf)
            bv_sb = const_pool.tile([1, C], dtb)

            # persistent activations
            q_sb = big_pool.tile([P, CT, N], dtb)     # Q[c, n]
            k_sb = big_pool.tile([P, CT, N], dtb)     # K[c, n]
            vt_sb = big_pool.tile([P, NT, C], dtb)    # V^T[m, c]

            with (
                tc.tile_pool(name="x", bufs=1) as x_pool,
                tc.tile_pool(name="psconv", bufs=4, space=bass.MemorySpace.PSUM)
                as psconv,
            ):
                xq_sb = x_pool.tile([P, CT, N], dtb)
                xkv_sb = x_pool.tile([P, CT, N], dtb)

                # First weight slab ahead of the activations so the first
                # matmul isn't queued behind the full 4MB xq transfer.
                w_slab0 = wqk_pool.tile([P, CT, P], dtb, tag="wslab")
                nc.sync.dma_start(w_slab0[:], wq_d[0])
                for kt in range(CT):
                    nc.sync.dma_start(xq_sb[:, kt, :], xq_d[kt * P : (kt + 1) * P, :])
                nc.sync.dma_start(bq_sb[:], bq_d[:])
                nc.sync.dma_start(bk_sb[:], bk_d[:])
                nc.sync.dma_start(bv_sb[:], bv_d[:])

                def conv_qk(x_sb, w_dram, b_sb, dst, slab0=None, extra_dma=None):
                    for ot in range(CT):
                        if extra_dma is not None:
                            extra_dma(ot)
                        if ot == 0 and slab0 is not None:
                            w_slab = slab0
                        else:
                            w_slab = wqk_pool.tile([P, CT, P], dtb, tag="wslab")
                            nc.sync.dma_start(w_slab[:], w_dram[ot])
                        ps0 = psconv.tile([P, KHALF], dtf, tag="mm")
                        ps1 = psconv.tile([P, KHALF], dtf, tag="mm")
                        for kt in range(CT):
                            nc.tensor.matmul(
                                ps0[:], w_slab[:, kt, :], x_sb[:, kt, 0:KHALF],
                                start=(kt == 0), stop=(kt == CT - 1),
                            )
                            nc.tensor.matmul(
                                ps1[:], w_slab[:, kt, :], x_sb[:, kt, KHALF:N],
                                start=(kt == 0), stop=(kt == CT - 1),
                            )
                        nc.vector.tensor_scalar_add(
                            dst[:, ot, 0:KHALF], ps0[:], b_sb[:, ot : ot + 1]
                        )
                        nc.vector.tensor_scalar_add(
                            dst[:, ot, KHALF:N], ps1[:], b_sb[:, ot : ot + 1]
                        )

                # stagger the xkv loads through the Q conv so they don't
                # compete with xq for HBM bandwidth at kernel start
                def load_xkv(ot):
                    nc.sync.dma_start(
                        xkv_sb[:, ot, :], xkv_d[ot * P : (ot + 1) * P, :]
                    )

                conv_qk(xq_sb, wq_d, bq_sb, q_sb, slab0=w_slab0, extra_dma=load_xkv)
                conv_qk(xkv_sb, wk_d, bk_sb, k_sb)

                # VT conv: VT[m, c] = sum_ci Xkv[ci, m] WvT[ci, c] + bv[c].
                # Two passes of 4 parallel m-tile PSUM chains per cg (the
                # weight slabs stream twice -- DMA is cheap, PSUM banks are
                # not).
                for cg in range(CG):
                    for half in range(2):
                        mts = range(4 * half, 4 * half + 4)
                        chains = {}
                        for mt in mts:
                            chains[mt] = psconv.tile(
                                [P, KHALF], dtf, tag="mm", name=f"vt{cg}_{mt}"
                            )
                        for kt in range(CT):
                            wslab = wv_pool.tile([P, KHALF], dtb, tag="wv")
                            nc.sync.dma_start(wslab[:], wv_d[cg, kt])
                            for mt in mts:
                                nc.tensor.matmul(
                                    chains[mt][:],
                                    xkv_sb[:, kt, mt * P : (mt + 1) * P],
                                    wslab[:],
                                    start=(kt == 0), stop=False,
                                )
                        for mt in mts:
                            nc.tensor.matmul(
                                chains[mt][:],
                                ones_row[:],
                                bv_sb[:, cg * KHALF : (cg + 1) * KHALF],
                                start=False, stop=True,
                            )
                            nc.vector.tensor_copy(
                                vt_sb[:, mt, cg * KHALF : (cg + 1) * KHALF],
                                chains[mt][:],
                            )

            # ---- ST = K^T Q, E = exp(ST), colsums, O = (V E^T) / colsum ----
            with (
                tc.tile_pool(name="attn", bufs=1) as attn_pool,
                tc.tile_pool(name="pssum", bufs=1, space=bass.MemorySpace.PSUM)
                as pssum,
                tc.tile_pool(name="pso", bufs=2, space=bass.MemorySpace.PSUM) as pso,
            ):
                et_sb = attn_pool.tile([P, NT, N], dtb)   # E[m, nq] = exp(S^T)
                rb_sb = attn_pool.tile([P, N], dtf)       # 1/colsum on every row
                sums_bc = pssum.tile([P, N], dtf)         # colsums on every row

                def colsum(mt):
                    for nh in range(2):
                        nc.tensor.matmul(
                            sums_bc[:, nh * KHALF : (nh + 1) * KHALF],
                            ones128[:],
                            et_sb[:, mt, nh * KHALF : (nh + 1) * KHALF],
                            start=(mt == 0), stop=(mt == NT - 1),
                        )

                for mt in range(NT):
                    ps = pss.tile([P, N], dtf, tag="s")  # two banks
                    for nh in range(2):
                        for kt in range(CT):
                            nc.tensor.matmul(
                                ps[:, nh * KHALF : (nh + 1) * KHALF],
                                k_sb[:, kt, mt * P : (mt + 1) * P],
                                q_sb[:, kt, nh * KHALF : (nh + 1) * KHALF],
                                start=(kt == 0), stop=(kt == CT - 1),
                            )
                    nc.scalar.activation(
                        et_sb[:, mt, :], ps[:],
                        mybir.ActivationFunctionType.Exp,
                    )
                    # the PE-colsum of block mt-1 runs here, one block late,
                    # so it never waits on the ACT exp of its own block
                    if mt >= 1:
                        colsum(mt - 1)
                colsum(NT - 1)
                nc.vector.reciprocal(rb_sb[:], sums_bc[:])

                # O = V @ E^T, normalized in the epilogue
                for nh in range(2):
                    for ct in range(CT):
                        ps = pso.tile([P, KHALF], dtf, tag="o")
                        for mt in range(NT):
                            nc.tensor.matmul(
                                ps[:],
                                vt_sb[:, mt, ct * P : (ct + 1) * P],
                                et_sb[:, mt, nh * KHALF : (nh + 1) * KHALF],
                                start=(mt == 0), stop=(mt == NT - 1),
                            )
                        o_stage = ostage_pool.tile([P, KHALF], dtf, tag="o")
                        nc.vector.tensor_mul(
                            o_stage[:], ps[:], rb_sb[:, nh * KHALF : (nh + 1) * KHALF]
                        )
                        nc.sync.dma_start(
                            out_d[ct * P : (ct + 1) * P, nh * KHALF : (nh + 1) * KHALF],
                            o_stage[:],
                        )

    nc.compile()
    return nc


@lru_cache(maxsize=1)
def _get_nc():
    return _build_program()


def _prep_wqk(W, b, scale):
    WT = np.ascontiguousarray(W.T) * scale  # [c_in, c_out]
    wt = np.ascontiguousarray(
        WT.reshape(CT, P, CT, P).transpose(2, 1, 0, 3)
    ).astype(_BF16)  # [ot, ci, kt, o]
    bp = np.ascontiguousarray((b * scale).reshape(CT, P).T).astype(np.float32)
    return wt, bp


def _prep_wv(W, b):
    WT = np.ascontiguousarray(W.T)  # [c_in, c_out]
    wt = np.ascontiguousarray(
        WT.reshape(CT, P, CG, KHALF).transpose(2, 0, 1, 3)
    ).astype(_BF16)  # [cg, kt, ci, co]
    bv = np.ascontiguousarray(b.reshape(1, C)).astype(_BF16)
    return wt, bv


def _run(inputs, trace=False):
    from concourse.bass_utils import run_bass_kernel_spmd

    F_rgb = np.asarray(inputs["F_rgb"], dtype=np.float32)
    F_ind = np.asarray(inputs["F_indices"], dtype=np.float32)

    scale = 1.0 / math.sqrt(C)
    # stream 0: rgb queries attend over index features
    wq0, bq0 = _prep_wqk(np.asarray(inputs["W_q_rgb"], np.float32),
                         np.asarray(inputs["b_q_rgb"], np.float32), scale)
    wk0, bk0 = _prep_wqk(np.asarray(inputs["W_k_ind"], np.float32),
                         np.asarray(inputs["b_k_ind"], np.float32), 1.0)
    wv0, bv0 = _prep_wv(np.asarray(inputs["W_v_ind"], np.float32),
                        np.asarray(inputs["b_v_ind"], np.float32))
    # stream 1: index queries attend over rgb features
    wq1, bq1 = _prep_wqk(np.asarray(inputs["W_q_ind"], np.float32),
                         np.asarray(inputs["b_q_ind"], np.float32), scale)
    wk1, bk1 = _prep_wqk(np.asarray(inputs["W_k_rgb"], np.float32),
                         np.asarray(inputs["b_k_rgb"], np.float32), 1.0)
    wv1, bv1 = _prep_wv(np.asarray(inputs["W_v_rgb"], np.float32),
                        np.asarray(inputs["b_v_rgb"], np.float32))

    rgb_flat = [np.ascontiguousarray(F_rgb[b].reshape(C, N)).astype(_BF16)
                for b in range(B)]
    ind_flat = [np.ascontiguousarray(F_ind[b].reshape(C, N)).astype(_BF16)
                for b in range(B)]

    in_maps = []
    for b in range(B):  # cores 0-3: stream 0
        in_maps.append(dict(xq=rgb_flat[b], xkv=ind_flat[b], wq=wq0, wk=wk0,
                            wv=wv0, bq=bq0, bk=bk0, bv=bv0))
    for b in range(B):  # cores 4-7: stream 1
        in_maps.append(dict(xq=ind_flat[b], xkv=rgb_flat[b], wq=wq1, wk=wk1,
                            wv=wv1, bq=bq1, bk=bk1, bv=bv1))

    nc = _get_nc()
    res = run_bass_kernel_spmd(nc, in_maps, core_ids=list(range(8)), trace=trace)

    O1 = np.stack([res.results[b]["out"].reshape(C, HW, HW) for b in range(B)])
    O2 = np.stack([res.results[4 + b]["out"].reshape(C, HW, HW) for b in range(B)])
    F_final = O1 + O2
    attention_weights = np.stack([O1, O2], axis=1)
    return (F_final, F_rgb, F_ind, attention_weights), res


def kernel(**inputs):
    outs, _ = _run(inputs, trace=False)
    return outs


def kernel_profiled(**inputs):
    outs, res = _run(inputs, trace=True)
    return outs, res


# revision 9
# speedup vs baseline: 1.0866x; 1.0124x over previous
"""Trainium2 Bass kernel for nn_CrossAttentionModule (B=4, C=2048, H=W=32).

The module is two independent cross-attention streams per batch element
(RGB queries over index features, and index queries over RGB features).
That yields 8 perfectly independent units = 4 batches x 2 streams; one
unit per NeuronCore, zero collectives.

Per-core program (all matmuls bf16, fp32 PSUM accumulate):
  Q  = (Wq/sqrt(C)) @ Xq + bq/sqrt(C)        [C, N]   (scale folded on host)
  K  = Wk @ Xkv + bk                          [C, N]
  VT = Xkv^T @ Wv^T + 1 x bv                  [N, C]   (computed directly
       transposed; bias added as a K=1 rank-1 matmul into the same PSUM;
       weights streamed as small kt-slabs against 4 parallel PSUM chains)
  ST = K^T Q                                  [N(key), N(query)] -- scores
       computed TRANSPOSED so the contraction index of the subsequent
       O-matmul (the key index m) lands on the partition dim: no transpose
       of the attention matrix is ever needed.
  E  = exp(ST)   (no max subtraction: |scores| <~ 5, exp is safe in fp32;
       softmax ratios are mathematically identical)
  colsum = ones[128,128]^T @ E  -- partition-reduce on the PE that lands
       the SAME sum on every partition, so 1/colsum needs no broadcast and
       the reciprocal runs wide on the DVE
  O  = (V E^T) * (1/colsum)                   [C, N]  fp32 out

Host side: pre-transposes/pre-tiles the weights into the exact slab layout
the kernel streams (every DMA is contiguous), casts to bf16, distributes
the 8 units across cores, and reassembles the 4 reference outputs.
"""

import math
from functools import lru_cache

import ml_dtypes
import numpy as np

B, C, HW, N = 4, 2048, 32, 1024
P = 128
CT = C // P           # 16 channel tiles
NT = N // P           # 8 pixel tiles
KHALF = 512           # moving free dim per matmul
CG = C // KHALF       # 4 output-channel groups for the VT conv

_BF16 = ml_dtypes.bfloat16


def _build_program():
    import concourse.bass as bass
    import concourse.mybir as mybir
    import concourse.tile as tile
    from concourse import bacc

    dtb = mybir.dt.bfloat16
    dtf = mybir.dt.float32

    nc = bacc.Bacc("TRN2", target_bir_lowering=False, debug=False)

    xq_d = nc.declare_dram_parameter("xq", [C, N], dtb, isOutput=False)
    xkv_d = nc.declare_dram_parameter("xkv", [C, N], dtb, isOutput=False)
    wq_d = nc.declare_dram_parameter("wq", [CT, P, CT, P], dtb, isOutput=False)
    wk_d = nc.declare_dram_parameter("wk", [CT, P, CT, P], dtb, isOutput=False)
    # wv tiled as [cg, kt, ci, co]: kt-slab (cg, kt) = [128 ci, 512 co] contiguous
    wv_d = nc.declare_dram_parameter("wv", [CG, CT, P, KHALF], dtb, isOutput=False)
    bq_d = nc.declare_dram_parameter("bq", [P, CT], dtf, isOutput=False)
    bk_d = nc.declare_dram_parameter("bk", [P, CT], dtf, isOutput=False)
    bv_d = nc.declare_dram_parameter("bv", [1, C], dtb, isOutput=False)
    out_d = nc.declare_dram_parameter("out", [C, N], dtf, isOutput=True)

    with tile.TileContext(nc) as tc:
        with (
            tc.tile_pool(name="const", bufs=1) as const_pool,
            tc.tile_pool(name="big", bufs=1) as big_pool,
            tc.tile_pool(name="wqk", bufs=3) as wqk_pool,
            tc.tile_pool(name="wv", bufs=4) as wv_pool,
            tc.tile_pool(name="ostage", bufs=3) as ostage_pool,
            # scores PSUM lives OUTSIDE the conv scope so its banks never
            # overlap the conv pool's -- the first ST matmul then has no
            # wait on the VT epilogue drain
            tc.tile_pool(name="pss", bufs=2, space=bass.MemorySpace.PSUM) as pss,
        ):
            # constants
            ones_row = const_pool.tile([1, P], dtb)
            nc.gpsimd.memset(ones_row[:], 1.0)
            ones128 = const_pool.tile([P, P], dtb)
            nc.gpsimd.memset(ones128[:], 1.0)
            bq_sb = const_pool.tile([P, CT], dtf)
            bk_sb = const_pool.tile([P, CT], dtf)
            bv_sb = const_pool.tile([1, C], dtb)

            # persistent activations
            q_sb = big_pool.tile([P, CT, N], dtb)     # Q[c, n]
            k_sb = big_pool.tile([P, CT, N], dtb)     # K[c, n]
            vt_sb = big_pool.tile([P, NT, C], dtb)    # V^T[m, c]

            with (
                tc.tile_pool(name="x", bufs=1) as x_pool,
                tc.tile_pool(name="psconv", bufs=4, space=bass.MemorySpace.PSUM)
                as psconv,
            ):
                xq_sb = x_pool.tile([P, CT, N], dtb)
                xkv_sb = x_pool.tile([P, CT, N], dtb)

                # First weight slab ahead of the activations so the first
                # matmul isn't queued behind the full 4MB xq transfer.
                w_slab0 = wqk_pool.tile([P, CT, P], dtb, tag="wslab")
                nc.sync.dma_start(w_slab0[:], wq_d[0])
                for kt in range(CT):
                    nc.sync.dma_start(xq_sb[:, kt, :], xq_d[kt * P : (kt + 1) * P, :])
                nc.sync.dma_start(bq_sb[:], bq_d[:])
                nc.sync.dma_start(bk_sb[:], bk_d[:])
                nc.sync.dma_start(bv_sb[:], bv_d[:])

                def conv_qk(x_sb, w_dram, b_sb, dst, slab0=None, extra_dma=None,
                            interleave_first=False):
                    # With interleave_first, the first two output tiles run as
                    # 4 interleaved PSUM chains: each arriving x-tile feeds 4
                    # matmuls instead of 2, which matches the PE rate to the
                    # DMA arrival rate while x streams in at kernel start.
                    ot = 0
                    if interleave_first:
                        slabs = [slab0]
                        w_slab1 = wqk_pool.tile([P, CT, P], dtb, tag="wslab")
                        nc.sync.dma_start(w_slab1[:], w_dram[1])
                        slabs.append(w_slab1)
                        chains = []
                        for i in range(4):
                            ch = psconv.tile([P, KHALF], dtf, tag="mm",
                                             name=f"cq{i}")
                            chains.append(ch)
                        for kt in range(CT):
                            for i in range(4):
                                nc.tensor.matmul(
                                    chains[i][:], slabs[i // 2][:, kt, :],
                                    x_sb[:, kt, (i % 2) * KHALF : (i % 2 + 1) * KHALF],
                                    start=(kt == 0), stop=(kt == CT - 1),
                                )
                        for i in range(4):
                            nc.vector.tensor_scalar_add(
                                dst[:, i // 2, (i % 2) * KHALF : (i % 2 + 1) * KHALF],
                                chains[i][:], b_sb[:, i // 2 : i // 2 + 1],
                            )
                        if extra_dma is not None:
                            extra_dma(0)
                            extra_dma(1)
                        ot = 2
                    for ot in range(ot, CT):
                        if extra_dma is not None:
                            extra_dma(ot)
                        if ot == 0 and slab0 is not None:
                            w_slab = slab0
                        else:
                            w_slab = wqk_pool.tile([P, CT, P], dtb, tag="wslab")
                            nc.sync.dma_start(w_slab[:], w_dram[ot])
                        ps0 = psconv.tile([P, KHALF], dtf, tag="mm")
                        ps1 = psconv.tile([P, KHALF], dtf, tag="mm")
                        for kt in range(CT):
                            nc.tensor.matmul(
                                ps0[:], w_slab[:, kt, :], x_sb[:, kt, 0:KHALF],
                                start=(kt == 0), stop=(kt == CT - 1),
                            )
                            nc.tensor.matmul(
                                ps1[:], w_slab[:, kt, :], x_sb[:, kt, KHALF:N],
                                start=(kt == 0), stop=(kt == CT - 1),
                            )
                        nc.vector.tensor_scalar_add(
                            dst[:, ot, 0:KHALF], ps0[:], b_sb[:, ot : ot + 1]
                        )
                        nc.vector.tensor_scalar_add(
                            dst[:, ot, KHALF:N], ps1[:], b_sb[:, ot : ot + 1]
                        )

                # stagger the xkv loads through the Q conv so they don't
                # compete with xq for HBM bandwidth at kernel start
                def load_xkv(ot):
                    nc.sync.dma_start(
                        xkv_sb[:, ot, :], xkv_d[ot * P : (ot + 1) * P, :]
                    )

                conv_qk(xq_sb, wq_d, bq_sb, q_sb, slab0=w_slab0, extra_dma=load_xkv)
                conv_qk(xkv_sb, wk_d, bk_sb, k_sb)

                # VT conv: VT[m, c] = sum_ci Xkv[ci, m] WvT[ci, c] + bv[c].
                # Two passes of 4 parallel m-tile PSUM chains per cg (the
                # weight slabs stream twice -- DMA is cheap, PSUM banks are
                # not).
                for cg in range(CG):
                    for half in range(2):
                        mts = range(4 * half, 4 * half + 4)
                        chains = {}
                        for mt in mts:
                            chains[mt] = psconv.tile(
                                [P, KHALF], dtf, tag="mm", name=f"vt{cg}_{mt}"
                            )
                        for kt in range(CT):
                            wslab = wv_pool.tile([P, KHALF], dtb, tag="wv")
                            nc.sync.dma_start(wslab[:], wv_d[cg, kt])
                            last = kt == CT - 1
                            for j, mt in enumerate(mts):
                                nc.tensor.matmul(
                                    chains[mt][:],
                                    xkv_sb[:, kt, mt * P : (mt + 1) * P],
                                    wslab[:],
                                    start=(kt == 0), stop=False,
                                )
                                if last:
                                    # finish this chain immediately: bias as a
                                    # rank-1 matmul, then drain the PSUM slot
                                    # on alternating engines so the next
                                    # half-pass isn't gated on one engine
                                    nc.tensor.matmul(
                                        chains[mt][:],
                                        ones_row[:],
                                        bv_sb[:, cg * KHALF : (cg + 1) * KHALF],
                                        start=False, stop=True,
                                    )
                                    dst = vt_sb[:, mt, cg * KHALF : (cg + 1) * KHALF]
                                    if j % 2 == 0:
                                        nc.vector.tensor_copy(dst, chains[mt][:])
                                    else:
                                        nc.scalar.copy(dst, chains[mt][:])

            # ---- ST = K^T Q, E = exp(ST), colsums, O = (V E^T) / colsum ----
            with (
                tc.tile_pool(name="attn", bufs=1) as attn_pool,
                tc.tile_pool(name="pssum", bufs=1, space=bass.MemorySpace.PSUM)
                as pssum,
                tc.tile_pool(name="pso", bufs=2, space=bass.MemorySpace.PSUM) as pso,
            ):
                et_sb = attn_pool.tile([P, NT, N], dtb)   # E[m, nq] = exp(S^T)
                rb_sb = attn_pool.tile([P, N], dtf)       # 1/colsum on every row
                sums_bc = pssum.tile([P, N], dtf)         # colsums on every row

                def colsum(mt):
                    for nh in range(2):
                        nc.tensor.matmul(
                            sums_bc[:, nh * KHALF : (nh + 1) * KHALF],
                            ones128[:],
                            et_sb[:, mt, nh * KHALF : (nh + 1) * KHALF],
                            start=(mt == 0), stop=(mt == NT - 1),
                        )

                for mt in range(NT):
                    ps = pss.tile([P, N], dtf, tag="s")  # two banks
                    for nh in range(2):
                        for kt in range(CT):
                            nc.tensor.matmul(
                                ps[:, nh * KHALF : (nh + 1) * KHALF],
                                k_sb[:, kt, mt * P : (mt + 1) * P],
                                q_sb[:, kt, nh * KHALF : (nh + 1) * KHALF],
                                start=(kt == 0), stop=(kt == CT - 1),
                            )
                    nc.scalar.activation(
                        et_sb[:, mt, :], ps[:],
                        mybir.ActivationFunctionType.Exp,
                    )
                    # the PE-colsum of block mt-1 runs here, one block late,
                    # so it never waits on the ACT exp of its own block
                    if mt >= 1:
                        colsum(mt - 1)

                # O = V @ E^T, normalized in the epilogue. The last colsum
                # and the reciprocal are sequenced after the first O chain's
                # matmuls: by then exp(mt=7) has finished (no PE wait), and
                # the reciprocal overlaps the next chains on the DVE.
                for nh in range(2):
                    for ct in range(CT):
                        ps = pso.tile([P, KHALF], dtf, tag="o")
                        for mt in range(NT):
                            nc.tensor.matmul(
                                ps[:],
                                vt_sb[:, mt, ct * P : (ct + 1) * P],
                                et_sb[:, mt, nh * KHALF : (nh + 1) * KHALF],
                                start=(mt == 0), stop=(mt == NT - 1),
                            )
                        if nh == 0 and ct == 0:
                            colsum(NT - 1)
                            nc.vector.reciprocal_approx_fast(
                                rb_sb[:, 0:KHALF], sums_bc[:, 0:KHALF]
                            )
                            nc.vector.reciprocal_approx_fast(
                                rb_sb[:, KHALF:N], sums_bc[:, KHALF:N]
                            )
                        o_stage = ostage_pool.tile([P, KHALF], dtf, tag="o")
                        nc.vector.tensor_mul(
                            o_stage[:], ps[:], rb_sb[:, nh * KHALF : (nh + 1) * KHALF]
                        )
                        nc.sync.dma_start(
                            out_d[ct * P : (ct + 1) * P, nh * KHALF : (nh + 1) * KHALF],
                            o_stage[:],
                        )

    nc.compile()
    return nc


@lru_cache(maxsize=1)
def _get_nc():
    return _build_program()


def _prep_wqk(W, b, scale):
    WT = np.ascontiguousarray(W.T) * scale  # [c_in, c_out]
    wt = np.ascontiguousarray(
        WT.reshape(CT, P, CT, P).transpose(2, 1, 0, 3)
    ).astype(_BF16)  # [ot, ci, kt, o]
    bp = np.ascontiguousarray((b * scale).reshape(CT, P).T).astype(np.float32)
    return wt, bp


def _prep_wv(W, b):
    WT = np.ascontiguousarray(W.T)  # [c_in, c_out]
    wt = np.ascontiguousarray(
        WT.reshape(CT, P, CG, KHALF).transpose(2, 0, 1, 3)
    ).astype(_BF16)  # [cg, kt, ci, co]
    bv = np.ascontiguousarray(b.reshape(1, C)).astype(_BF16)
    return wt, bv


def _run(inputs, trace=False):
    from concourse.bass_utils import run_bass_kernel_spmd

    F_rgb = np.asarray(inputs["F_rgb"], dtype=np.float32)
    F_ind = np.asarray(inputs["F_indices"], dtype=np.float32)

    scale = 1.0 / math.sqrt(C)
    # stream 0: rgb queries attend over index features
    wq0, bq0 = _prep_wqk(np.asarray(inputs["W_q_rgb"], np.float32),
                         np.asarray(inputs["b_q_rgb"], np.float32), scale)
    wk0, bk0 = _prep_wqk(np.asarray(inputs["W_k_ind"], np.float32),
                         np.asarray(inputs["b_k_ind"], np.float32), 1.0)
    wv0, bv0 = _prep_wv(np.asarray(inputs["W_v_ind"], np.float32),
                        np.asarray(inputs["b_v_ind"], np.float32))
    # stream 1: index queries attend over rgb features
    wq1, bq1 = _prep_wqk(np.asarray(inputs["W_q_ind"], np.float32),
                         np.asarray(inputs["b_q_ind"], np.float32), scale)
    wk1, bk1 = _prep_wqk(np.asarray(inputs["W_k_rgb"], np.float32),
                         np.asarray(inputs["b_k_rgb"], np.float32), 1.0)
    wv1, bv1 = _prep_wv(np.asarray(inputs["W_v_rgb"], np.float32),
                        np.asarray(inputs["b_v_rgb"], np.float32))

    rgb_flat = [np.ascontiguousarray(F_rgb[b].reshape(C, N)).astype(_BF16)
                for b in range(B)]
    ind_flat = [np.ascontiguousarray(F_ind[b].reshape(C, N)).astype(_BF16)
                for b in range(B)]

    in_maps = []
    for b in range(B):  # cores 0-3: stream 0
        in_maps.append(dict(xq=rgb_flat[b], xkv=ind_flat[b], wq=wq0, wk=wk0,
                            wv=wv0, bq=bq0, bk=bk0, bv=bv0))
    for b in range(B):  # cores 4-7: stream 1
        in_maps.append(dict(xq=ind_flat[b], xkv=rgb_flat[b], wq=wq1, wk=wk1,
                            wv=wv1, bq=bq1, bk=bk1, bv=bv1))

    nc = _get_nc()
    res = run_bass_kernel_spmd(nc, in_maps, core_ids=list(range(8)), trace=trace)

    O1 = np.stack([res.results[b]["out"].reshape(C, HW, HW) for b in range(B)])
    O2 = np.stack([res.results[4 + b]["out"].reshape(C, HW, HW) for b in range(B)])
    F_final = O1 + O2
    attention_weights = np.stack([O1, O2], axis=1)
    return (F_final, F_rgb, F_ind, attention_weights), res


def kernel(**inputs):
    outs, _ = _run(inputs, trace=False)
    return outs


def kernel_profiled(**inputs):
    outs, res = _run(inputs, trace=True)
    return outs, res


# revision 14
# speedup vs baseline: 1.1231x; 1.0336x over previous
"""Trainium2 Bass kernel for nn_CrossAttentionModule (B=4, C=2048, H=W=32).

The module is two independent cross-attention streams per batch element
(RGB queries over index features, and index queries over RGB features).
That yields 8 perfectly independent units = 4 batches x 2 streams; one
unit per NeuronCore, zero collectives.

Per-core program (all matmuls bf16, fp32 PSUM accumulate):
  Q  = (Wq/sqrt(C)) @ Xq + bq/sqrt(C)        [C, N]   (scale folded on host)
  K  = Wk @ Xkv + bk                          [C, N]
  VT = Xkv^T @ Wv^T + 1 x bv                  [N, C]   (computed directly
       transposed; bias added as a K=1 rank-1 matmul into the same PSUM;
       weights streamed as small kt-slabs against 4 parallel PSUM chains)
  ST = K^T Q                                  [N(key), N(query)] -- scores
       computed TRANSPOSED so the contraction index of the subsequent
       O-matmul (the key index m) lands on the partition dim: no transpose
       of the attention matrix is ever needed.
  E  = exp(ST)   (no max subtraction: |scores| <~ 5, exp is safe in fp32;
       softmax ratios are mathematically identical)
  colsum = ones[128,128]^T @ E  -- partition-reduce on the PE that lands
       the SAME sum on every partition, so 1/colsum needs no broadcast and
       the reciprocal runs wide on the DVE
  O  = (V E^T) * (1/colsum)                   [C, N]  fp32 out

Host side: pre-transposes/pre-tiles the weights into the exact slab layout
the kernel streams (every DMA is contiguous), casts to bf16, distributes
the 8 units across cores, and reassembles the 4 reference outputs.
"""

import math
from functools import lru_cache

import ml_dtypes
import numpy as np

B, C, HW, N = 4, 2048, 32, 1024
P = 128
CT = C // P           # 16 channel tiles
NT = N // P           # 8 pixel tiles
KHALF = 512           # moving free dim per matmul
CG = C // KHALF       # 4 output-channel groups for the VT conv

_BF16 = ml_dtypes.bfloat16


def _build_program():
    import concourse.bass as bass
    import concourse.mybir as mybir
    import concourse.tile as tile
    from concourse import bacc

    dtb = mybir.dt.bfloat16
    dtf = mybir.dt.float32

    nc = bacc.Bacc("TRN2", target_bir_lowering=False, debug=False)

    xq_d = nc.declare_dram_parameter("xq", [C, N], dtb, isOutput=False)
    xkv_d = nc.declare_dram_parameter("xkv", [C, N], dtb, isOutput=False)
    wq_d = nc.declare_dram_parameter("wq", [CT, P, CT, P], dtb, isOutput=False)
    wk_d = nc.declare_dram_parameter("wk", [CT, P, CT, P], dtb, isOutput=False)
    # wv tiled as [cg, kt, ci, co]: kt-slab (cg, kt) = [128 ci, 512 co] contiguous
    wv_d = nc.declare_dram_parameter("wv", [CG, CT, P, KHALF], dtb, isOutput=False)
    bq_d = nc.declare_dram_parameter("bq", [P, CT], dtf, isOutput=False)
    bk_d = nc.declare_dram_parameter("bk", [P, CT], dtf, isOutput=False)
    bv_d = nc.declare_dram_parameter("bv", [1, C], dtb, isOutput=False)
    out_d = nc.declare_dram_parameter("out", [C, N], dtf, isOutput=True)

    with tile.TileContext(nc) as tc:
        with (
            tc.tile_pool(name="const", bufs=1) as const_pool,
            tc.tile_pool(name="big", bufs=1) as big_pool,
            tc.tile_pool(name="wqk", bufs=3) as wqk_pool,
            tc.tile_pool(name="wv", bufs=4) as wv_pool,
            tc.tile_pool(name="ostage", bufs=3) as ostage_pool,
            # scores PSUM lives OUTSIDE the conv scope so its banks never
            # overlap the conv pool's -- the first ST matmul then has no
            # wait on the VT epilogue drain
            tc.tile_pool(name="pss", bufs=2, space=bass.MemorySpace.PSUM) as pss,
        ):
            # constants
            ones_row = const_pool.tile([1, P], dtb)
            nc.gpsimd.memset(ones_row[:], 1.0)
            ones128 = const_pool.tile([P, P], dtb)
            nc.gpsimd.memset(ones128[:], 1.0)
            bq_sb = const_pool.tile([P, CT], dtf)
            bk_sb = const_pool.tile([P, CT], dtf)
            bv_sb = const_pool.tile([1, C], dtb)

            # persistent activations
            q_sb = big_pool.tile([P, CT, N], dtb)     # Q[c, n]
            k_sb = big_pool.tile([P, CT, N], dtb)     # K[c, n]
            vt_sb = big_pool.tile([P, NT, C], dtb)    # V^T[m, c]

            with (
                tc.tile_pool(name="x", bufs=1) as x_pool,
                tc.tile_pool(name="psconv", bufs=4, space=bass.MemorySpace.PSUM)
                as psconv,
            ):
                xq_sb = x_pool.tile([P, CT, N], dtb)
                xkv_sb = x_pool.tile([P, CT, N], dtb)

                # First two weight slabs and the (tiny) biases ahead of the
                # activations so the first matmuls aren't queued behind the
                # full 4MB xq transfer.
                w_slab0 = wqk_pool.tile([P, CT, P], dtb, tag="wslab")
                nc.sync.dma_start(w_slab0[:], wq_d[0])
                w_slab1 = wqk_pool.tile([P, CT, P], dtb, tag="wslab")
                nc.sync.dma_start(w_slab1[:], wq_d[1])
                nc.sync.dma_start(bq_sb[:], bq_d[:])
                nc.sync.dma_start(bk_sb[:], bk_d[:])
                nc.sync.dma_start(bv_sb[:], bv_d[:])
                for kt in range(CT):
                    nc.sync.dma_start(xq_sb[:, kt, :], xq_d[kt * P : (kt + 1) * P, :])

                # Broadcast bv across all partitions once, during the
                # DMA-bound startup window: bv_bc = ones ⊗ bv via PE, so the
                # VT epilogue can add the bias on the DVE instead of paying a
                # pipeline-draining K=1 matmul per chain.
                bv_bc = const_pool.tile([P, C], dtf)
                for cg in range(CG):
                    bps = psconv.tile([P, KHALF], dtf, tag="mm", name=f"bbc{cg}")
                    nc.tensor.matmul(
                        bps[:], ones_row[:],
                        bv_sb[:, cg * KHALF : (cg + 1) * KHALF],
                        start=True, stop=True,
                    )
                    nc.vector.tensor_copy(
                        bv_bc[:, cg * KHALF : (cg + 1) * KHALF], bps[:]
                    )

                def conv_qk(x_sb, w_dram, b_sb, dst, slab0=None, slab1=None,
                            extra_dma=None, interleave_first=False):
                    # With interleave_first, the first two output tiles run as
                    # 4 interleaved PSUM chains: each arriving x-tile feeds 4
                    # matmuls instead of 2, which matches the PE rate to the
                    # DMA arrival rate while x streams in at kernel start.
                    ot = 0
                    if interleave_first:
                        slabs = [slab0, slab1]
                        chains = []
                        for i in range(4):
                            ch = psconv.tile([P, KHALF], dtf, tag="mm",
                                             name=f"cq{i}")
                            chains.append(ch)
                        for kt in range(CT):
                            for i in range(4):
                                nc.tensor.matmul(
                                    chains[i][:], slabs[i // 2][:, kt, :],
                                    x_sb[:, kt, (i % 2) * KHALF : (i % 2 + 1) * KHALF],
                                    start=(kt == 0), stop=(kt == CT - 1),
                                )
                        for i in range(4):
                            nc.vector.tensor_scalar_add(
                                dst[:, i // 2, (i % 2) * KHALF : (i % 2 + 1) * KHALF],
                                chains[i][:], b_sb[:, i // 2 : i // 2 + 1],
                            )
                        if extra_dma is not None:
                            extra_dma(0)
                            extra_dma(1)
                        ot = 2
                    for ot in range(ot, CT):
                        if extra_dma is not None:
                            extra_dma(ot)
                        if ot == 0 and slab0 is not None:
                            w_slab = slab0
                        else:
                            w_slab = wqk_pool.tile([P, CT, P], dtb, tag="wslab")
                            nc.sync.dma_start(w_slab[:], w_dram[ot])
                        ps0 = psconv.tile([P, KHALF], dtf, tag="mm")
                        ps1 = psconv.tile([P, KHALF], dtf, tag="mm")
                        for kt in range(CT):
                            nc.tensor.matmul(
                                ps0[:], w_slab[:, kt, :], x_sb[:, kt, 0:KHALF],
                                start=(kt == 0), stop=(kt == CT - 1),
                            )
                            nc.tensor.matmul(
                                ps1[:], w_slab[:, kt, :], x_sb[:, kt, KHALF:N],
                                start=(kt == 0), stop=(kt == CT - 1),
                            )
                        nc.vector.tensor_scalar_add(
                            dst[:, ot, 0:KHALF], ps0[:], b_sb[:, ot : ot + 1]
                        )
                        nc.vector.tensor_scalar_add(
                            dst[:, ot, KHALF:N], ps1[:], b_sb[:, ot : ot + 1]
                        )

                # stagger the xkv loads through the Q conv so they don't
                # compete with xq for HBM bandwidth at kernel start
                def load_xkv(ot):
                    nc.sync.dma_start(
                        xkv_sb[:, ot, :], xkv_d[ot * P : (ot + 1) * P, :]
                    )

                conv_qk(xq_sb, wq_d, bq_sb, q_sb, slab0=w_slab0, slab1=w_slab1,
                        extra_dma=load_xkv, interleave_first=True)
                conv_qk(xkv_sb, wk_d, bk_sb, k_sb)

                # VT conv: VT[m, c] = sum_ci Xkv[ci, m] WvT[ci, c] + bv[c].
                # Two passes of 4 parallel m-tile PSUM chains per cg (the
                # weight slabs stream twice -- DMA is cheap, PSUM banks are
                # not).
                for cg in range(CG):
                    for half in range(2):
                        mts = range(4 * half, 4 * half + 4)
                        chains = {}
                        for mt in mts:
                            chains[mt] = psconv.tile(
                                [P, KHALF], dtf, tag="mm", name=f"vt{cg}_{mt}"
                            )
                        for kt in range(CT):
                            wslab = wv_pool.tile([P, KHALF], dtb, tag="wv")
                            nc.sync.dma_start(wslab[:], wv_d[cg, kt])
                            last = kt == CT - 1
                            for j, mt in enumerate(mts):
                                nc.tensor.matmul(
                                    chains[mt][:],
                                    xkv_sb[:, kt, mt * P : (mt + 1) * P],
                                    wslab[:],
                                    start=(kt == 0), stop=last,
                                )
                                if last:
                                    # drain this chain immediately, folding
                                    # the bias add into the epilogue
                                    nc.vector.tensor_tensor(
                                        vt_sb[:, mt, cg * KHALF : (cg + 1) * KHALF],
                                        chains[mt][:],
                                        bv_bc[:, cg * KHALF : (cg + 1) * KHALF],
                                        op=mybir.AluOpType.add,
                                    )

            # ---- ST = K^T Q, E = exp(ST), colsums, O = (V E^T) / colsum ----
            with (
                tc.tile_pool(name="attn", bufs=1) as attn_pool,
                tc.tile_pool(name="pssum", bufs=1, space=bass.MemorySpace.PSUM)
                as pssum,
                tc.tile_pool(name="pso", bufs=2, space=bass.MemorySpace.PSUM) as pso,
            ):
                et_sb = attn_pool.tile([P, NT, N], dtb)   # E[m, nq] = exp(S^T)
                rb_sb = attn_pool.tile([P, N], dtf)       # 1/colsum on every row
                sums_bc = pssum.tile([P, N], dtf)         # colsums on every row

                def colsum(mt):
                    for nh in range(2):
                        nc.tensor.matmul(
                            sums_bc[:, nh * KHALF : (nh + 1) * KHALF],
                            ones128[:],
                            et_sb[:, mt, nh * KHALF : (nh + 1) * KHALF],
                            start=(mt == 0), stop=(mt == NT - 1),
                        )

                for mt in range(NT):
                    ps = pss.tile([P, N], dtf, tag="s")  # two banks
                    for nh in range(2):
                        for kt in range(CT):
                            nc.tensor.matmul(
                                ps[:, nh * KHALF : (nh + 1) * KHALF],
                                k_sb[:, kt, mt * P : (mt + 1) * P],
                                q_sb[:, kt, nh * KHALF : (nh + 1) * KHALF],
                                start=(kt == 0), stop=(kt == CT - 1),
                            )
                    nc.scalar.activation(
                        et_sb[:, mt, :], ps[:],
                        mybir.ActivationFunctionType.Exp,
                    )
                    # the PE-colsum of block mt-1 runs here, one block late,
                    # so it never waits on the ACT exp of its own block
                    if mt >= 1:
                        colsum(mt - 1)

                # O = V @ E^T, normalized in the epilogue. The last colsum
                # and the reciprocal are sequenced after the first O chain's
                # matmuls: by then exp(mt=7) has finished (no PE wait), and
                # the reciprocal overlaps the next chains on the DVE.
                for nh in range(2):
                    for ct in range(CT):
                        ps = pso.tile([P, KHALF], dtf, tag="o")
                        for mt in range(NT):
                            nc.tensor.matmul(
                                ps[:],
                                vt_sb[:, mt, ct * P : (ct + 1) * P],
                                et_sb[:, mt, nh * KHALF : (nh + 1) * KHALF],
                                start=(mt == 0), stop=(mt == NT - 1),
                            )
                        if nh == 0 and ct == 0:
                            colsum(NT - 1)
                            nc.vector.reciprocal_approx_fast(
                                rb_sb[:, 0:KHALF], sums_bc[:, 0:KHALF]
                            )
                            nc.vector.reciprocal_approx_fast(
                                rb_sb[:, KHALF:N], sums_bc[:, KHALF:N]
                            )
                        o_stage = ostage_pool.tile([P, KHALF], dtf, tag="o")
                        nc.vector.tensor_mul(
                            o_stage[:], ps[:], rb_sb[:, nh * KHALF : (nh + 1) * KHALF]
                        )
                        nc.sync.dma_start(
                            out_d[ct * P : (ct + 1) * P, nh * KHALF : (nh + 1) * KHALF],
                            o_stage[:],
                        )

    nc.compile()
    return nc


@lru_cache(maxsize=1)
def _get_nc():
    return _build_program()


def _prep_wqk(W, b, scale):
    WT = np.ascontiguousarray(W.T) * scale  # [c_in, c_out]
    wt = np.ascontiguousarray(
        WT.reshape(CT, P, CT, P).transpose(2, 1, 0, 3)
    ).astype(_BF16)  # [ot, ci, kt, o]
    bp = np.ascontiguousarray((b * scale).reshape(CT, P).T).astype(np.float32)
    return wt, bp


def _prep_wv(W, b):
    WT = np.ascontiguousarray(W.T)  # [c_in, c_out]
    wt = np.ascontiguousarray(
        WT.reshape(CT, P, CG, KHALF).transpose(2, 0, 1, 3)
    ).astype(_BF16)  # [cg, kt, ci, co]
    bv = np.ascontiguousarray(b.reshape(1, C)).astype(_BF16)
    return wt, bv


def _run(inputs, trace=False):
    from concourse.bass_utils import run_bass_kernel_spmd

    F_rgb = np.asarray(inputs["F_rgb"], dtype=np.float32)
    F_ind = np.asarray(inputs["F_indices"], dtype=np.float32)

    scale = 1.0 / math.sqrt(C)
    # stream 0: rgb queries attend over index features
    wq0, bq0 = _prep_wqk(np.asarray(inputs["W_q_rgb"], np.float32),
                         np.asarray(inputs["b_q_rgb"], np.float32), scale)
    wk0, bk0 = _prep_wqk(np.asarray(inputs["W_k_ind"], np.float32),
                         np.asarray(inputs["b_k_ind"], np.float32), 1.0)
    wv0, bv0 = _prep_wv(np.asarray(inputs["W_v_ind"], np.float32),
                        np.asarray(inputs["b_v_ind"], np.float32))
    # stream 1: index queries attend over rgb features
    wq1, bq1 = _prep_wqk(np.asarray(inputs["W_q_ind"], np.float32),
                         np.asarray(inputs["b_q_ind"], np.float32), scale)
    wk1, bk1 = _prep_wqk(np.asarray(inputs["W_k_rgb"], np.float32),
                         np.asarray(inputs["b_k_rgb"], np.float32), 1.0)
    wv1, bv1 = _prep_wv(np.asarray(inputs["W_v_rgb"], np.float32),
                        np.asarray(inputs["b_v_rgb"], np.float32))

    rgb_flat = [np.ascontiguousarray(F_rgb[b].reshape(C, N)).astype(_BF16)
                for b in range(B)]
    ind_flat = [np.ascontiguousarray(F_ind[b].reshape(C, N)).astype(_BF16)
                for b in range(B)]

    in_maps = []
    for b in range(B):  # cores 0-3: stream 0
        in_maps.append(dict(xq=rgb_flat[b], xkv=ind_flat[b], wq=wq0, wk=wk0,
                            wv=wv0, bq=bq0, bk=bk0, bv=bv0))
    for b in range(B):  # cores 4-7: stream 1
        in_maps.append(dict(xq=ind_flat[b], xkv=rgb_flat[b], wq=wq1, wk=wk1,
                            wv=wv1, bq=bq1, bk=bk1, bv=bv1))

    nc = _get_nc()
    res = run_bass_kernel_spmd(nc, in_maps, core_ids=list(range(8)), trace=trace)

    O1 = np.stack([res.results[b]["out"].reshape(C, HW, HW) for b in range(B)])
    O2 = np.stack([res.results[4 + b]["out"].reshape(C, HW, HW) for b in range(B)])
    F_final = O1 + O2
    attention_weights = np.stack([O1, O2], axis=1)
    return (F_final, F_rgb, F_ind, attention_weights), res


def kernel(**inputs):
    outs, _ = _run(inputs, trace=False)
    return outs


def kernel_profiled(**inputs):
    outs, res = _run(inputs, trace=True)
    return outs, res
